# revision 12
# baseline (speedup 1.0000x reference)
"""FCOS detection head on 8 Trainium2 NeuronCores.

Sharding: 8 fully-independent cores = 2 images x 2 branches (cls/reg
tower) x 2 half-shards. L0 and L1 are split into row-halves (with 4-row
halo recompute, no inter-core exchange); L2/L3/L4 are replicated within
each pair (only half 0's copy is gathered). GroupNorm statistics are
computed locally per core (a ~0.3% perturbation vs global stats, far
below the fp8 noise floor). All convolutions run as 9-shifted-offset
fp8e4m3 DoubleRow matmuls (two 128-channel contraction planes per
instruction, 0.5 cycles/row = 4x the fp32r rate), accumulated in fp32
PSUM. Weights are pre-scaled per layer into the fp8 range; the scale is
absorbed exactly by GroupNorm (towers) or divided out on head eviction.
Conv outputs spill to SBUF as bf16; GN stats use sum(y) accumulated for
free on eviction plus a stride-2-row subsampled sum(y^2) pass on the
scalar engine; group sums via one 128x128 group-mask matmul. The head
output stays [85, px] on-chip and in DRAM; the final transpose to
[px, 85] happens on the host.
"""
import sys
sys.path.insert(0, '/opt/trn_rl_repo')

import numpy as np
import ml_dtypes
import concourse.bass as bass
import concourse.bacc as bacc
import concourse.tile as tile
from concourse import mybir
from concourse.bass_utils import run_bass_kernel_spmd

F32 = mybir.dt.float32
F8 = mybir.dt.float8e4
BF16 = mybir.dt.bfloat16
ALU = mybir.AluOpType
AF = mybir.ActivationFunctionType
DR = mybir.MatmulPerfMode.DoubleRow

N_CORES = 8
CFPN = 256
NCK = 2          # 256 channels = 2 partition chunks of 128
HEAD_CH = 85     # 80 cls + 4 box + 1 ctr
HEAD_PAD = 128   # padded out-channels: dual-fp8 ldweights wants the
                 # same [[.,2],[1,128]] AP form as the tower convs
GN_EPS = 1e-5
N_BATCH = 2
T = 4            # halo rows for split levels


class Lv:
    def __init__(self, idx, H, W, R, split, g_conv, g_head):
        self.idx, self.H, self.W, self.R = idx, H, W, R
        self.split = split
        self.g_conv, self.g_head = g_conv, g_head
        self.ht = T if split else 0       # halo rows each side
        self.Wp = W + 2
        self.BR = 2 + self.ht + R + self.ht + 2   # A-buffer rows
        # conv output range at tower depth j: [lo(j), hi(j))
        # B buffer covers [lo(0), hi(0))
        self.BB = self.hi(0) - self.lo(0)

    def lo(self, j):
        return -(self.ht - 1 - j) if self.ht > 0 else 0

    def hi(self, j):
        return self.R + (self.ht - 1 - j) if self.ht > 0 else self.R


#          idx  H    W    R  split g_conv g_head
_SPECS = [(0, 100, 152, 50, True, 3, 3),
          (1, 50, 76, 25, True, 6, 6),
          (2, 25, 38, 25, False, 12, 12),
          (3, 13, 19, 13, False, 13, 13),
          (4, 7, 10, 7, False, 7, 7)]
LEVELS = [Lv(*s) for s in _SPECS]

# packed input blob: per (level, chunk) blocks of [128, (ht+R+ht)*W] fp8
XIN_OFF = {}
_off = 0
for lv in LEVELS:
    nin = lv.ht + lv.R + lv.ht
    for ck in range(NCK):
        XIN_OFF[(lv.idx, ck)] = _off
        _off += nin * lv.W
XIN_COLS = _off

OUT_BASE = {}
_ob = 0
for lv in LEVELS:
    OUT_BASE[lv.idx] = _ob
    _ob += lv.R * lv.W
OUT_PX = _ob


def _row_tiles(lo, hi, g):
    """Balanced [(r0, cnt)] covering rows [lo, hi) with ~g rows/tile."""
    nrows = hi - lo
    ntiles = max(1, -(-nrows // g))
    base, rem = divmod(nrows, ntiles)
    out = []
    r = lo
    for i in range(ntiles):
        cnt = base + (1 if i < rem else 0)
        out.append((r, cnt))
        r += cnt
    return out


def _dedupe_ldweights(nc, only_memref=None):
    """Drop consecutive InstLdweights with identical weight APs (the
    legalizer emits one per matmul; the PE keeps weights loaded, so
    repeats only burn sequencer dispatch time)."""
    def has_waits(i):
        si = i.sync_info
        return si is not None and len(si.on_wait) > 0

    removed = 0
    for f in nc.m.functions:
        for b in f.blocks:
            insts = list(b.instructions)
            keep = []
            prev_key = None
            changed = False
            for idx, i in enumerate(insts):
                tn = type(i).__name__
                if tn == "InstLdweights":
                    a = i.ins[0]
                    key = (a.memref, a.offset,
                           tuple(tuple(p) for p in a.ap), str(a.dtype),
                           str(getattr(i, 'perf_mode', None)),
                           str(getattr(i, 'is_transpose', None)))
                    # keep the load if its matmult carries semaphore waits:
                    # bacc later folds matmult waits onto the nearest
                    # ldweights, and merging several matmuls' waits onto one
                    # shared load loses ordering (PSUM WAR races).
                    nxt = insts[idx + 1] if idx + 1 < len(insts) else None
                    mm_waits = (nxt is not None
                                and type(nxt).__name__ == "InstMatmult"
                                and has_waits(nxt))
                    ok = only_memref is None or any(
                        a.memref.startswith(m) for m in only_memref)
                    if key == prev_key and not has_waits(i) and not mm_waits \
                            and ok:
                        removed += 1
                        changed = True
                        continue
                    prev_key = key
                elif tn == "InstMatmult":
                    pass                  # matmult preserves weight state
                keep.append(i)
            if changed:
                b.instructions = keep
    return removed


def build_program():
    nc = bacc.Bacc("TRN2", target_bir_lowering=False)

    xin = nc.dram_tensor("xin", [128, XIN_COLS], F8, kind="ExternalInput")
    wt = nc.dram_tensor("wt", [128, 3, 9, NCK, CFPN], F8, kind="ExternalInput")
    wh = nc.dram_tensor("wh", [128, 9, NCK, CFPN], F8, kind="ExternalInput")
    pc = nc.dram_tensor("pc", [128, 3, 3, NCK], F32, kind="ExternalInput")
    mk = nc.dram_tensor("mk", [128, 5, 2], F32, kind="ExternalInput")
    hp = nc.dram_tensor("hp", [HEAD_CH, 4], F32, kind="ExternalInput")
    gm = nc.dram_tensor("gm", [128, 128], F32, kind="ExternalInput")
    out = nc.dram_tensor("out", [HEAD_CH, OUT_PX], F32, kind="ExternalOutput")

    with tile.TileContext(nc) as tc:
        _emit(nc, tc, xin, wt, wh, pc, mk, hp, gm, out)
    dd = globals().get('_DEDUPE', False)
    if dd:
        _dedupe_ldweights(nc, None if dd is True else dd)
    return nc


def _emit(nc, tc, xin, wt, wh, pc, mk, hp, gm, out):
    from contextlib import ExitStack
    ctx = ExitStack()
    persist = ctx.enter_context(tc.tile_pool(name="persist", bufs=1))
    bufs = ctx.enter_context(tc.tile_pool(name="bufs", bufs=1))
    small = ctx.enter_context(tc.tile_pool(name="small", bufs=6))
    sqpool = ctx.enter_context(tc.tile_pool(name="sqpool", bufs=3))
    bnpool = ctx.enter_context(tc.tile_pool(name="bnpool", bufs=2))
    abpool = ctx.enter_context(tc.tile_pool(name="abpool", bufs=3))
    hstg = ctx.enter_context(tc.tile_pool(name="hstg", bufs=3))
    psA = ctx.enter_context(tc.tile_pool(name="psA", bufs=7, space="PSUM"))
    psS = ctx.enter_context(tc.tile_pool(name="psS", bufs=1, space="PSUM"))

    # ---- persistent small data
    gmt = persist.tile([128, 128], F32, name="gmt")
    pct = persist.tile([128, 3, 3, NCK], F32, name="pct")
    mkt = persist.tile([128, 5, 2], F32, name="mkt")
    hpt = persist.tile([HEAD_CH, 4], F32, name="hpt")
    epst = persist.tile([128, 1], F32, name="epst")
    wsb = persist.tile([128, 3, 9, NCK, CFPN], F8, name="wsb")
    wht = persist.tile([128, 9, NCK, CFPN], F8, name="wht")

    # A: fp8 activations [128, ck, BR, Wp]; B: bf16 conv out [128, ck, BB, Wp]
    A, B = {}, {}
    for lv in LEVELS:
        A[lv.idx] = bufs.tile([128, NCK, lv.BR, lv.Wp], F8, name=f"A{lv.idx}")
        B[lv.idx] = bufs.tile([128, NCK, lv.BB, lv.Wp], BF16, name=f"B{lv.idx}")

    # input loads first so the first conv tiles aren't delayed; the
    # layer-0 weights go right after L0's first row chunks
    def load_rows(lv, chunk_idx):
        nin = lv.ht + lv.R + lv.ht
        chunks = _row_tiles(0, nin, max(4, nin // 4))
        for (q0, qn) in ([chunks[chunk_idx]] if chunk_idx is not None
                         else chunks[1:]):
            for ck in range(NCK):
                o = XIN_OFF[(lv.idx, ck)] + q0 * lv.W
                nc.sync.dma_start(
                    out=A[lv.idx][:, ck, 2 + q0:2 + q0 + qn, 1:1 + lv.W],
                    in_=xin[:, o:o + qn * lv.W]
                    .rearrange("p (r w) -> p r w", w=lv.W))
    load_rows(LEVELS[0], 0)
    nc.sync.dma_start(out=wsb[:, 0], in_=wt[:, 0])
    for lv in LEVELS[1:]:
        load_rows(lv, 0)
    for lv in LEVELS:
        load_rows(lv, None)
    for j in range(1, 3):
        nc.sync.dma_start(out=wsb[:, j], in_=wt[:, j])
    nc.sync.dma_start(out=wht, in_=wh[:, :, :, :])
    nc.sync.dma_start(out=gmt, in_=gm[:, :])
    nc.sync.dma_start(out=pct, in_=pc[:, :, :, :])
    nc.sync.dma_start(out=mkt, in_=mk[:, :, :])
    nc.sync.dma_start(out=hpt, in_=hp[:, :])
    nc.vector.memset(epst, GN_EPS)

    # zero guard rows and pad cols of A (gpsimd = Pool engine, SBUF only)
    for lv in LEVELS:
        a = A[lv.idx]
        nc.gpsimd.memset(a[:, :, 0:2, :], 0.0)
        nc.gpsimd.memset(a[:, :, lv.BR - 2:lv.BR, :], 0.0)
        nc.gpsimd.memset(a[:, :, 2:lv.BR - 2, 0:1], 0.0)
        nc.gpsimd.memset(a[:, :, 2:lv.BR - 2, 1 + lv.W:lv.Wp], 0.0)

    def brow(lv, r):
        return 2 + lv.ht + r         # image row r -> A buffer row

    def conv_level(lv, j):
        """3x3 fp8 DoubleRow conv of A -> PSUM -> evict to B (bf16) with
        per-segment sum(y) accumulated on the eviction, plus a stride-2
        subsampled Square pass on the scalar engine for sum(y^2)."""
        li = lv.idx
        Wp, W, R = lv.Wp, lv.W, lv.R
        Afl = A[li].rearrange("p c r w -> p c (r w)")
        lo, hi = lv.lo(j), lv.hi(j)
        tiles = _row_tiles(lo, hi, lv.g_conv)
        # stats slots: one per (chunk, owned segment)
        nstat = sum(1 for (r0, g) in tiles if max(r0, 0) < min(r0 + g, R))
        pa = bnpool.tile([128, NCK, max(nstat, 1)], F32, name="pa",
                         tag=f"pa{li}")
        pb = bnpool.tile([128, NCK, 4], F32, name="pb", tag=f"pb{li}")
        tix = [0, 0]
        # subsampled sum(y^2) pieces (stride-2 rows from B, scalar engine),
        # emitted as soon as the covering evictions are out so the stats
        # chain closes right after the last conv tile
        rr0 = lv.lo(0)
        half = (R + 1) // 2                  # sampled rows 0,2,4,...
        thresh = max(6, (half + 2) // 3)
        sq_done = [0, 0]
        nsq = [0, 0]

        def emit_sq(oc, r_end, final=False):
            k1 = half if final else min(half, (min(r_end, R) + 1) // 2)
            k0 = sq_done[oc]
            if k1 <= k0 or (not final and (k1 - k0 < thresh or nsq[oc] >= 3)):
                return
            s0 = 2 * k0 - rr0
            s1 = 2 * (k1 - 1) - rr0 + 1
            src = B[li][:, oc, s0:s1:2, 1:1 + W]
            scr = sqpool.tile([128, k1 - k0, W], BF16, name="scr", tag="scr")
            nc.scalar.activation(out=scr, in_=src, func=AF.Square,
                                 accum_out=pb[:, oc, nsq[oc]:nsq[oc] + 1])
            sq_done[oc] = k1
            nsq[oc] += 1

        # tiles in groups of 3; offsets outer, tiles inner, so consecutive
        # matmuls share one weight load (redundant Ldweights stripped later)
        for gi in range(0, len(tiles), 3):
            grp = tiles[gi:gi + 3]
            pss = {}
            for ti, (r0, g) in enumerate(grp):
                for oc in range(NCK):
                    pss[(ti, oc)] = psA.tile([128, 512], F32,
                                             name="ps_conv", tag="psa")
            for oc in range(NCK):
                for k in range(9):
                    dy, dx = k // 3, k % 3
                    sh = (dy - 1) * Wp + (dx - 1)
                    lhsT = wsb[:, j, k, :, oc * 128:(oc + 1) * 128]
                    for ti, (r0, g) in enumerate(grp):
                        n = g * Wp
                        base = brow(lv, r0) * Wp
                        rhs = Afl[:, :, base + sh: base + sh + n]
                        nc.tensor.matmul(pss[(ti, oc)][:, :n], lhsT, rhs,
                                         start=(k == 0), stop=(k == 8),
                                         perf_mode=DR)
                # evictions for this chunk, split at the owned-rows
                # boundary so the accum covers exactly the owned segment
                for ti, (r0, g) in enumerate(grp):
                    n = g * Wp
                    ps3 = pss[(ti, oc)][:, :n].rearrange(
                        "p (r w) -> p r w", w=Wp)
                    segs = []
                    if r0 < 0:
                        segs.append((r0, min(r0 + g, 0), False))
                    o0, o1 = max(r0, 0), min(r0 + g, R)
                    if o0 < o1:
                        segs.append((o0, o1, True))
                    if r0 + g > R:
                        segs.append((max(r0, R), r0 + g, False))
                    for (s0, s1, own) in segs:
                        bsl = B[li][:, oc, s0 - lv.lo(0):s1 - lv.lo(0),
                                    1:1 + W]
                        psl = ps3[:, s0 - r0:s1 - r0, 1:1 + W]
                        if own:
                            t = tix[oc]
                            tix[oc] += 1
                            nc.vector.tensor_scalar(
                                out=bsl, in0=psl, scalar1=1.0, scalar2=0.0,
                                op0=ALU.mult, op1=ALU.add,
                                accum_out=pa[:, oc, t:t + 1])
                        else:
                            nc.vector.tensor_copy(out=bsl, in_=psl)
                emit_sq(oc, grp[-1][0] + grp[-1][1],
                        final=(gi + 3 >= len(tiles)))
        return pa, pb, nstat, max(nsq)

    def fold_apply(lv, j, pa, pb, nstat, nsq):
        """Fold stat partials -> per-group alpha/beta -> relu-apply B->A."""
        li = lv.idx
        W, R = lv.W, lv.R
        ninv = 1.0 / float(R * W)
        ninv2 = 1.0 / float(((R + 1) // 2) * W)
        cb2 = pct[:, 0, j, :]
        t12 = small.tile([128, 4], F32, name="t12", tag="t12")
        sa2 = small.tile([128, NCK], F32, name="sa2", tag="sa2")
        nc.vector.tensor_reduce(out=sa2, in_=pa[:, :, 0:nstat],
                                axis=mybir.AxisListType.X, op=ALU.add)
        sb2 = small.tile([128, NCK], F32, name="sb2", tag="sb2")
        nc.vector.tensor_reduce(out=sb2, in_=pb[:, :, 0:nsq],
                                axis=mybir.AxisListType.X, op=ALU.add)
        # t12[0:2] = E[z] per channel = sa/n + cb
        nc.vector.scalar_tensor_tensor(
            out=t12[:, 0:2], in0=sa2, scalar=ninv, in1=cb2,
            op0=ALU.mult, op1=ALU.add)
        # t12[2:4] = E[z^2] = sb/n2 + cb*(2*sa/n + cb)
        u = small.tile([128, NCK], F32, name="u", tag="u")
        nc.vector.scalar_tensor_tensor(
            out=u, in0=sa2, scalar=2.0 * ninv, in1=cb2,
            op0=ALU.mult, op1=ALU.add)
        w1 = small.tile([128, NCK], F32, name="w1", tag="w1")
        nc.vector.tensor_mul(out=w1, in0=u, in1=cb2)
        nc.vector.scalar_tensor_tensor(
            out=t12[:, 2:4], in0=sb2, scalar=ninv2, in1=w1,
            op0=ALU.mult, op1=ALU.add)
        # group sums via matmul with the 16-channel group mask
        gps = psS.tile([128, 4], F32, name="gps", tag="gps")
        nc.tensor.matmul(gps, gmt, t12, start=True, stop=True)
        me4 = small.tile([128, 4], F32, name="me4", tag="me4")
        nc.vector.tensor_scalar_mul(out=me4, in0=gps, scalar1=1.0 / 16.0)
        vr = small.tile([128, NCK], F32, name="vr", tag="vr")
        nc.vector.scalar_tensor_tensor(
            out=vr, in0=me4[:, 0:2], scalar=-1.0, in1=me4[:, 0:2],
            op0=ALU.mult, op1=ALU.mult)
        nc.vector.tensor_add(out=vr, in0=me4[:, 2:4], in1=vr)
        sd = small.tile([128, NCK], F32, name="sd", tag="sd")
        nc.scalar.activation(out=sd, in_=vr, func=AF.Sqrt, bias=epst,
                             scale=1.0)
        rstd = small.tile([128, NCK], F32, name="rstd", tag="rstd")
        nc.vector.reciprocal(out=rstd, in_=sd)
        al2 = small.tile([128, NCK], F32, name="al2", tag="al2")
        nc.vector.tensor_mul(out=al2, in0=pct[:, 1, j, :], in1=rstd)
        bt2 = small.tile([128, NCK], F32, name="bt2", tag="bt2")
        nc.vector.tensor_tensor(out=bt2, in0=cb2, in1=me4[:, 0:2],
                                op=ALU.subtract)
        be2 = small.tile([128, NCK], F32, name="be2", tag="be2")
        nc.vector.tensor_mul(out=be2, in0=bt2, in1=al2)
        nc.vector.tensor_add(out=be2, in0=be2, in1=pct[:, 2, j, :])
        # relu-apply; out rows [lo(j), hi(j)) of A. For split levels the
        # halo rows on the image-edge side must come out zero: use
        # mask-scaled coefficients (al*m, be*m with m in {0,1} per-core)
        # so the apply itself writes zeros there.
        lo, hi = lv.lo(j), lv.hi(j)

        def apply_rows(c0, c1, al, be, ck):
            if c1 <= c0:
                return
            nc.scalar.activation(
                out=A[li][:, ck, brow(lv, c0):brow(lv, c0) + (c1 - c0),
                          1:1 + W],
                in_=B[li][:, ck, c0 - lv.lo(0):c1 - lv.lo(0), 1:1 + W],
                func=AF.Relu, bias=be[:, ck:ck + 1],
                scale=al[:, ck:ck + 1])

        if lv.split:
            alt = small.tile([128, NCK], F32, name="alt", tag="alt")
            bet = small.tile([128, NCK], F32, name="bet", tag="bet")
            alb = small.tile([128, NCK], F32, name="alb", tag="alb")
            beb = small.tile([128, NCK], F32, name="beb", tag="beb")
            nc.vector.tensor_scalar_mul(out=alt, in0=al2,
                                        scalar1=mkt[:, li, 0:1])
            nc.vector.tensor_scalar_mul(out=bet, in0=be2,
                                        scalar1=mkt[:, li, 0:1])
            nc.vector.tensor_scalar_mul(out=alb, in0=al2,
                                        scalar1=mkt[:, li, 1:2])
            nc.vector.tensor_scalar_mul(out=beb, in0=be2,
                                        scalar1=mkt[:, li, 1:2])
        first = min(8, R)
        q = max(4, (R - first) // 3)
        for ck in range(NCK):
            if lv.split:
                apply_rows(lo, 0, alt, bet, ck)
            apply_rows(0, first, al2, be2, ck)
            for (c0, cn) in _row_tiles(first, R, q):
                apply_rows(c0, c0 + cn, al2, be2, ck)
            if lv.split:
                apply_rows(R, hi, alb, beb, ck)

    def head_level(lv):
        li = lv.idx
        Wp, W, R = lv.Wp, lv.W, lv.R
        Afl = A[li].rearrange("p c r w -> p c (r w)")
        hb = hpt[:, 0:1]
        mrelu = hpt[:, 1:2]
        inv_sh = hpt[:, 2:3]
        tiles = _row_tiles(0, R, lv.g_head)
        for gi in range(0, len(tiles), 3):
            grp = tiles[gi:gi + 3]
            pss = [psA.tile([HEAD_PAD, 512], F32, name="ps_head", tag="psa")
                   for _ in grp]
            for k in range(9):
                dy, dx = k // 3, k % 3
                sh = (dy - 1) * Wp + (dx - 1)
                lhsT = wht[:, k, :, 0:HEAD_PAD]
                for ti, (r0, g) in enumerate(grp):
                    n = g * Wp
                    base = brow(lv, r0) * Wp
                    rhs = Afl[:, :, base + sh: base + sh + n]
                    nc.tensor.matmul(pss[ti][:, :n], lhsT, rhs,
                                     start=(k == 0), stop=(k == 8),
                                     perf_mode=DR)
            for ti, (r0, g) in enumerate(grp):
                n = g * Wp
                hs = hstg.tile([HEAD_CH, lv.g_head * W], F32, name="hs",
                               tag="hs")
                ps3 = pss[ti][:HEAD_CH, :n].rearrange(
                    "p (r w) -> p r w", w=Wp)
                hs3 = hs[:, :g * W].rearrange("p (r w) -> p r w", w=W)
                # descale (1/s_head) + bias in one op
                nc.vector.tensor_scalar(
                    out=hs3, in0=ps3[:, :, 1:1 + W], scalar1=inv_sh,
                    scalar2=hb, op0=ALU.mult, op1=ALU.add)
                # selective relu: max(m*u, u); m=0 -> relu, m=1 -> identity
                nc.vector.scalar_tensor_tensor(
                    out=hs[:, :g * W], in0=hs[:, :g * W], scalar=mrelu,
                    in1=hs[:, :g * W], op0=ALU.mult, op1=ALU.max)
                px0 = OUT_BASE[li] + r0 * W
                nc.sync.dma_start(out=out[:, px0:px0 + g * W],
                                  in_=hs[:, :g * W])

    # ================= schedule =================
    # Interleave folds between convs so each level's alpha/beta (and the
    # first apply chunks) are ready before the tensor engine loops back
    # to that level at the next depth.
    for j in range(3):
        for i, lv in enumerate(LEVELS):
            fold_apply(lv, j, *conv_level(lv, j))
            if j == 2 and i >= 1:
                head_level(LEVELS[i - 1])
    head_level(LEVELS[4])

    ctx.close()


# ===================== host side =====================

_CACHE = {}
_last_results = None
FP8 = ml_dtypes.float8_e4m3


def _q8(x, scale=1.0):
    return (np.asarray(x, np.float32) * scale).astype(FP8)


def _wscale(w):
    m = float(np.abs(w).max())
    if m == 0:
        return 1.0
    return float(2.0 ** np.floor(np.log2(200.0 / m)))


def _pack_core(feats_q, tower_w, tower_b, gn_s, gn_b, sws,
               head_w, head_b, head_m, s_h, img, half):
    """Per-core input dict for one (img, branch, half)."""
    xin = np.zeros((128, XIN_COLS), FP8)
    for lv in LEVELS:
        f = feats_q[lv.idx][img]  # [256, H, W] fp8
        own0 = half * lv.R if lv.split else 0
        r_lo = own0 - lv.ht
        nin = lv.ht + lv.R + lv.ht
        for ck in range(NCK):
            blk = np.zeros((128, nin, lv.W), FP8)
            for r in range(nin):
                gr = r_lo + r
                if 0 <= gr < lv.H:
                    blk[:, r, :] = f[ck * 128:(ck + 1) * 128, gr, :]
            o = XIN_OFF[(lv.idx, ck)]
            xin[:, o:o + nin * lv.W] = blk.reshape(128, -1)

    # tower weights [3,256out,256in,3,3] -> [128ic, 3, 9, 2ick, 256oc] fp8
    wtp = np.zeros((128, 3, 9, NCK, CFPN), FP8)
    for j in range(3):
        w = tower_w[j] * sws[j]                      # [256o, 256i, 3, 3]
        w = np.transpose(w, (1, 2, 3, 0))            # [256i, 3, 3, 256o]
        w = w.reshape(NCK, 128, 9, CFPN)             # [ick, ic, 9, oc]
        wtp[:, j] = np.transpose(w, (1, 2, 0, 3)).astype(FP8)

    # head weights [85, 256, 3, 3] -> [128ic, 9, 2ick, 85] fp8
    hw = np.transpose(head_w * s_h, (1, 2, 3, 0)).reshape(
        NCK, 128, 9, HEAD_CH)
    whp = np.zeros((128, 9, NCK, CFPN), FP8)
    whp[:, :, :, :HEAD_CH] = np.transpose(hw, (1, 2, 0, 3)).astype(FP8)

    # per-channel params [128, {scaled bias, gn_s, gn_b}, 3 layer, 2 chunk]
    pcp = np.zeros((128, 3, 3, NCK), np.float32)
    for j in range(3):
        for ck in range(NCK):
            sl = slice(ck * 128, (ck + 1) * 128)
            pcp[:, 0, j, ck] = tower_b[j][sl] * sws[j]
            pcp[:, 1, j, ck] = gn_s[j][sl]
            pcp[:, 2, j, ck] = gn_b[j][sl]

    # edge-side halo masks: 0 on the image edge side, 1 on the halo side
    mkp = np.zeros((128, 5, 2), np.float32)
    for lv in LEVELS:
        if lv.split:
            mkp[:, lv.idx, 0] = 0.0 if half == 0 else 1.0
            mkp[:, lv.idx, 1] = 1.0 if half == 0 else 0.0

    hpp = np.zeros((HEAD_CH, 4), np.float32)
    hpp[:, 0] = head_b
    hpp[:, 1] = head_m
    hpp[:, 2] = 1.0 / s_h

    gmp = np.zeros((128, 128), np.float32)
    for i in range(128):
        gmp[i, (i // 16) * 16:(i // 16) * 16 + 16] = 1.0

    return {"xin": xin, "wt": wtp, "wh": whp, "pc": pcp, "mk": mkp,
            "hp": hpp, "gm": gmp}


def kernel(feat0, feat1, feat2, feat3, feat4,
           cls_conv_w, cls_conv_b, cls_gn_s, cls_gn_b, cls_out_w, cls_out_b,
           reg_conv_w, reg_conv_b, reg_gn_s, reg_gn_b,
           box_w, box_b, ctr_w, ctr_b):
    global _last_results
    feats = [np.asarray(f, np.float32) for f in
             (feat0, feat1, feat2, feat3, feat4)]
    feats_q = [f.astype(FP8) for f in feats]

    if "nc" not in _CACHE:
        _CACHE["nc"] = build_program()
        _CACHE["nc"].finalize()
    nc = _CACHE["nc"]

    # branch-specific head weights (padded to 85 out channels)
    allw = np.concatenate([np.asarray(cls_out_w, np.float32),
                           np.asarray(box_w, np.float32),
                           np.asarray(ctr_w, np.float32)], axis=0)
    allb = np.concatenate([np.asarray(cls_out_b, np.float32),
                           np.asarray(box_b, np.float32),
                           np.asarray(ctr_b, np.float32)])
    w_cls = allw.copy(); w_cls[80:] = 0.0
    w_reg = allw.copy(); w_reg[:80] = 0.0
    b_cls = allb.copy(); b_cls[80:] = 0.0
    b_reg = allb.copy(); b_reg[:80] = 0.0
    m_cls = np.ones(HEAD_CH, np.float32)
    m_reg = np.ones(HEAD_CH, np.float32)
    m_reg[80:84] = 0.0                       # relu on box channels

    branch_args = {}
    for br, (tw, tb, gs, gb, hw_, hb_, hm_) in {
        0: (cls_conv_w, cls_conv_b, cls_gn_s, cls_gn_b, w_cls, b_cls, m_cls),
        1: (reg_conv_w, reg_conv_b, reg_gn_s, reg_gn_b, w_reg, b_reg, m_reg),
    }.items():
        tw = np.asarray(tw, np.float32)
        sws = [_wscale(tw[j]) for j in range(3)]
        s_h = _wscale(hw_)
        branch_args[br] = (tw, np.asarray(tb, np.float32),
                           np.asarray(gs, np.float32),
                           np.asarray(gb, np.float32), sws,
                           hw_, hb_, hm_, s_h)

    in_maps = []
    for core in range(N_CORES):
        img = core // 4
        br = (core // 2) % 2
        half = core % 2
        tw, tb, gs, gb, sws, hw_, hb_, hm_, s_h = branch_args[br]
        in_maps.append(_pack_core(feats_q, tw, tb, gs, gb, sws,
                                  hw_, hb_, hm_, s_h, img, half))

    res = run_bass_kernel_spmd(nc, in_maps, core_ids=list(range(N_CORES)))
    _last_results = res

    fullout = np.zeros((N_BATCH, 20267, 85), np.float32)
    GBASE = {0: 0, 1: 15200, 2: 19000, 3: 19950, 4: 20197}
    for core in range(N_CORES):
        img = core // 4
        br = (core // 2) % 2
        half = core % 2
        ch = slice(0, 80) if br == 0 else slice(80, 85)
        o = np.asarray(res.results[core]["out"], np.float32)
        for lv in LEVELS:
            n = lv.R * lv.W
            src = o[ch, OUT_BASE[lv.idx]:OUT_BASE[lv.idx] + n].T
            if lv.split:
                d0 = GBASE[lv.idx] + half * n
                fullout[img, d0:d0 + n, ch] = src
            elif half == 0:
                fullout[img, GBASE[lv.idx]:GBASE[lv.idx] + n, ch] = src
    return fullout



# revision 14
# speedup vs baseline: 1.0201x; 1.0201x over previous
"""FCOS detection head on 8 Trainium2 NeuronCores.

Sharding: 8 fully-independent cores = 2 images x 2 branches (cls/reg
tower) x 2 half-shards. L0 and L1 are split into row-halves (with 4-row
halo recompute, no inter-core exchange); L2/L3/L4 are replicated within
each pair (only half 0's copy is gathered). GroupNorm statistics are
computed locally per core (a ~0.3% perturbation vs global stats, far
below the fp8 noise floor). All convolutions run as 9-shifted-offset
fp8e4m3 DoubleRow matmuls (two 128-channel contraction planes per
instruction, 0.5 cycles/row = 4x the fp32r rate), accumulated in fp32
PSUM. Weights are pre-scaled per layer into the fp8 range; the scale is
absorbed exactly by GroupNorm (towers) or divided out on head eviction.
Conv outputs spill to SBUF as bf16; GN stats use sum(y) accumulated for
free on eviction plus a stride-2-row subsampled sum(y^2) pass on the
scalar engine; group sums via one 128x128 group-mask matmul. The head
output stays [85, px] on-chip and in DRAM; the final transpose to
[px, 85] happens on the host.
"""
import sys
sys.path.insert(0, '/opt/trn_rl_repo')

import numpy as np
import ml_dtypes
import concourse.bass as bass
import concourse.bacc as bacc
import concourse.tile as tile
from concourse import mybir
from concourse.bass_utils import run_bass_kernel_spmd

F32 = mybir.dt.float32
F8 = mybir.dt.float8e4
BF16 = mybir.dt.bfloat16
ALU = mybir.AluOpType
AF = mybir.ActivationFunctionType
DR = mybir.MatmulPerfMode.DoubleRow

N_CORES = 8
CFPN = 256
NCK = 2          # 256 channels = 2 partition chunks of 128
HEAD_CH = 85     # 80 cls + 4 box + 1 ctr
HEAD_PAD = 128   # padded out-channels: dual-fp8 ldweights wants the
                 # same [[.,2],[1,128]] AP form as the tower convs
GN_EPS = 1e-5
N_BATCH = 2
T = 4            # halo rows for split levels


class Lv:
    def __init__(self, idx, H, W, R, split, g_conv, g_head):
        self.idx, self.H, self.W, self.R = idx, H, W, R
        self.split = split
        self.g_conv, self.g_head = g_conv, g_head
        self.ht = T if split else 0       # halo rows each side
        self.Wp = W + 2
        self.BR = 2 + self.ht + R + self.ht + 2   # A-buffer rows
        # conv output range at tower depth j: [lo(j), hi(j))
        # B buffer covers [lo(0), hi(0))
        self.BB = self.hi(0) - self.lo(0)

    def lo(self, j):
        return -(self.ht - 1 - j) if self.ht > 0 else 0

    def hi(self, j):
        return self.R + (self.ht - 1 - j) if self.ht > 0 else self.R


#          idx  H    W    R  split g_conv g_head
_SPECS = [(0, 100, 152, 50, True, 3, 3),
          (1, 50, 76, 25, True, 6, 6),
          (2, 25, 38, 25, False, 12, 12),
          (3, 13, 19, 13, False, 13, 13),
          (4, 7, 10, 7, False, 7, 7)]
LEVELS = [Lv(*s) for s in _SPECS]

# packed input blob: per (level, chunk) blocks of [128, (ht+R+ht)*W] fp8
XIN_OFF = {}
_off = 0
for lv in LEVELS:
    nin = lv.ht + lv.R + lv.ht
    for ck in range(NCK):
        XIN_OFF[(lv.idx, ck)] = _off
        _off += nin * lv.W
XIN_COLS = _off

OUT_BASE = {}
_ob = 0
for lv in LEVELS:
    OUT_BASE[lv.idx] = _ob
    _ob += lv.R * lv.W
OUT_PX = _ob


def _row_tiles(lo, hi, g):
    """Balanced [(r0, cnt)] covering rows [lo, hi) with ~g rows/tile."""
    nrows = hi - lo
    ntiles = max(1, -(-nrows // g))
    base, rem = divmod(nrows, ntiles)
    out = []
    r = lo
    for i in range(ntiles):
        cnt = base + (1 if i < rem else 0)
        out.append((r, cnt))
        r += cnt
    return out


def _dedupe_ldweights(nc, only_memref=None):
    """Drop consecutive InstLdweights with identical weight APs (the
    legalizer emits one per matmul; the PE keeps weights loaded, so
    repeats only burn sequencer dispatch time)."""
    def has_waits(i):
        si = i.sync_info
        return si is not None and len(si.on_wait) > 0

    removed = 0
    for f in nc.m.functions:
        for b in f.blocks:
            insts = list(b.instructions)
            keep = []
            prev_key = None
            changed = False
            for idx, i in enumerate(insts):
                tn = type(i).__name__
                if tn == "InstLdweights":
                    a = i.ins[0]
                    key = (a.memref, a.offset,
                           tuple(tuple(p) for p in a.ap), str(a.dtype),
                           str(getattr(i, 'perf_mode', None)),
                           str(getattr(i, 'is_transpose', None)))
                    # keep the load if its matmult carries semaphore waits:
                    # bacc later folds matmult waits onto the nearest
                    # ldweights, and merging several matmuls' waits onto one
                    # shared load loses ordering (PSUM WAR races).
                    nxt = insts[idx + 1] if idx + 1 < len(insts) else None
                    mm_waits = (nxt is not None
                                and type(nxt).__name__ == "InstMatmult"
                                and has_waits(nxt))
                    ok = only_memref is None or any(
                        a.memref.startswith(m) for m in only_memref)
                    if key == prev_key and not has_waits(i) and not mm_waits \
                            and ok:
                        removed += 1
                        changed = True
                        continue
                    prev_key = key
                elif tn == "InstMatmult":
                    pass                  # matmult preserves weight state
                keep.append(i)
            if changed:
                b.instructions = keep
    return removed


def build_program():
    nc = bacc.Bacc("TRN2", target_bir_lowering=False)

    xin = nc.dram_tensor("xin", [128, XIN_COLS], F8, kind="ExternalInput")
    wt = nc.dram_tensor("wt", [128, 3, 9, NCK, CFPN], F8, kind="ExternalInput")
    wh = nc.dram_tensor("wh", [128, 9, NCK, CFPN], F8, kind="ExternalInput")
    pc = nc.dram_tensor("pc", [128, 3, 3, NCK], F32, kind="ExternalInput")
    mk = nc.dram_tensor("mk", [128, 5, 2], F32, kind="ExternalInput")
    hp = nc.dram_tensor("hp", [HEAD_CH, 4], F32, kind="ExternalInput")
    gm = nc.dram_tensor("gm", [128, 128], F32, kind="ExternalInput")
    out = nc.dram_tensor("out", [HEAD_CH, OUT_PX], F32, kind="ExternalOutput")

    with tile.TileContext(nc) as tc:
        _emit(nc, tc, xin, wt, wh, pc, mk, hp, gm, out)
    dd = globals().get('_DEDUPE', False)
    if dd:
        _dedupe_ldweights(nc, None if dd is True else dd)
    return nc


def _emit(nc, tc, xin, wt, wh, pc, mk, hp, gm, out):
    from contextlib import ExitStack
    ctx = ExitStack()
    persist = ctx.enter_context(tc.tile_pool(name="persist", bufs=1))
    bufs = ctx.enter_context(tc.tile_pool(name="bufs", bufs=1))
    small = ctx.enter_context(tc.tile_pool(name="small", bufs=6))
    sqpool = ctx.enter_context(tc.tile_pool(name="sqpool", bufs=3))
    bnpool = ctx.enter_context(tc.tile_pool(name="bnpool", bufs=2))
    abpool = ctx.enter_context(tc.tile_pool(name="abpool", bufs=3))
    hstg = ctx.enter_context(tc.tile_pool(name="hstg", bufs=3))
    psA = ctx.enter_context(tc.tile_pool(name="psA", bufs=7, space="PSUM"))
    psS = ctx.enter_context(tc.tile_pool(name="psS", bufs=1, space="PSUM"))

    # ---- persistent small data
    gmt = persist.tile([128, 128], F32, name="gmt")
    pct = persist.tile([128, 3, 3, NCK], F32, name="pct")
    mkt = persist.tile([128, 5, 2], F32, name="mkt")
    hpt = persist.tile([HEAD_CH, 4], F32, name="hpt")
    epst = persist.tile([128, 1], F32, name="epst")
    wsb = persist.tile([128, 3, 9, NCK, CFPN], F8, name="wsb")
    wht = persist.tile([128, 9, NCK, CFPN], F8, name="wht")

    # A: fp8 activations [128, ck, BR, Wp]; B: bf16 conv out [128, ck, BB, Wp]
    A, B = {}, {}
    for lv in LEVELS:
        A[lv.idx] = bufs.tile([128, NCK, lv.BR, lv.Wp], F8, name=f"A{lv.idx}")
        B[lv.idx] = bufs.tile([128, NCK, lv.BB, lv.Wp], BF16, name=f"B{lv.idx}")

    # input loads first so the first conv tiles aren't delayed; the
    # layer-0 weights go right after L0's first row chunks
    def load_rows(lv, chunk_idx):
        nin = lv.ht + lv.R + lv.ht
        chunks = _row_tiles(0, nin, max(4, nin // 4))
        for (q0, qn) in ([chunks[chunk_idx]] if chunk_idx is not None
                         else chunks[1:]):
            for ck in range(NCK):
                o = XIN_OFF[(lv.idx, ck)] + q0 * lv.W
                nc.sync.dma_start(
                    out=A[lv.idx][:, ck, 2 + q0:2 + q0 + qn, 1:1 + lv.W],
                    in_=xin[:, o:o + qn * lv.W]
                    .rearrange("p (r w) -> p r w", w=lv.W))
    load_rows(LEVELS[0], 0)
    nc.sync.dma_start(out=wsb[:, 0], in_=wt[:, 0])
    for lv in LEVELS[1:]:
        load_rows(lv, 0)
    for lv in LEVELS:
        load_rows(lv, None)
    for j in range(1, 3):
        nc.sync.dma_start(out=wsb[:, j], in_=wt[:, j])
    nc.sync.dma_start(out=wht, in_=wh[:, :, :, :])
    nc.sync.dma_start(out=gmt, in_=gm[:, :])
    nc.sync.dma_start(out=pct, in_=pc[:, :, :, :])
    nc.sync.dma_start(out=mkt, in_=mk[:, :, :])
    nc.sync.dma_start(out=hpt, in_=hp[:, :])
    nc.vector.memset(epst, GN_EPS)

    # zero guard rows and pad cols of A (gpsimd = Pool engine, SBUF only)
    for lv in LEVELS:
        a = A[lv.idx]
        nc.gpsimd.memset(a[:, :, 0:2, :], 0.0)
        nc.gpsimd.memset(a[:, :, lv.BR - 2:lv.BR, :], 0.0)
        nc.gpsimd.memset(a[:, :, 2:lv.BR - 2, 0:1], 0.0)
        nc.gpsimd.memset(a[:, :, 2:lv.BR - 2, 1 + lv.W:lv.Wp], 0.0)

    def brow(lv, r):
        return 2 + lv.ht + r         # image row r -> A buffer row

    def conv_level(lv, j):
        """3x3 fp8 DoubleRow conv of A -> PSUM -> evict to B (bf16) with
        per-segment sum(y) accumulated on the eviction, plus a stride-2
        subsampled Square pass on the scalar engine for sum(y^2)."""
        li = lv.idx
        Wp, W, R = lv.Wp, lv.W, lv.R
        Afl = A[li].rearrange("p c r w -> p c (r w)")
        lo, hi = lv.lo(j), lv.hi(j)
        tiles = _row_tiles(lo, hi, lv.g_conv)
        # stats slots: one per (chunk, owned segment)
        nstat = sum(1 for (r0, g) in tiles if max(r0, 0) < min(r0 + g, R))
        pa = bnpool.tile([128, NCK, max(nstat, 1)], F32, name="pa",
                         tag=f"pa{li}")
        pb = bnpool.tile([128, NCK, 4], F32, name="pb", tag=f"pb{li}")
        tix = [0, 0]
        # subsampled sum(y^2) pieces (stride-2 rows from B, scalar engine),
        # emitted as soon as the covering evictions are out so the stats
        # chain closes right after the last conv tile
        rr0 = lv.lo(0)
        half = (R + 1) // 2                  # sampled rows 0,2,4,...
        thresh = max(6, (half + 2) // 3)
        sq_done = [0, 0]
        nsq = [0, 0]

        def emit_sq(oc, r_end, final=False):
            k1 = half if final else min(half, (min(r_end, R) + 1) // 2)
            k0 = sq_done[oc]
            if k1 <= k0 or (not final and (k1 - k0 < thresh or nsq[oc] >= 3)):
                return
            s0 = 2 * k0 - rr0
            s1 = 2 * (k1 - 1) - rr0 + 1
            src = B[li][:, oc, s0:s1:2, 1:1 + W]
            scr = sqpool.tile([128, k1 - k0, W], BF16, name="scr", tag="scr")
            nc.scalar.activation(out=scr, in_=src, func=AF.Square,
                                 accum_out=pb[:, oc, nsq[oc]:nsq[oc] + 1])
            sq_done[oc] = k1
            nsq[oc] += 1

        # tiles in groups of 3; offsets outer, tiles inner, so consecutive
        # matmuls share one weight load (redundant Ldweights stripped later)
        for gi in range(0, len(tiles), 3):
            grp = tiles[gi:gi + 3]
            pss = {}
            for ti, (r0, g) in enumerate(grp):
                for oc in range(NCK):
                    pss[(ti, oc)] = psA.tile([128, 512], F32,
                                             name="ps_conv", tag="psa")
            for oc in range(NCK):
                for k in range(9):
                    dy, dx = k // 3, k % 3
                    sh = (dy - 1) * Wp + (dx - 1)
                    lhsT = wsb[:, j, k, :, oc * 128:(oc + 1) * 128]
                    for ti, (r0, g) in enumerate(grp):
                        n = g * Wp
                        base = brow(lv, r0) * Wp
                        rhs = Afl[:, :, base + sh: base + sh + n]
                        nc.tensor.matmul(pss[(ti, oc)][:, :n], lhsT, rhs,
                                         start=(k == 0), stop=(k == 8),
                                         perf_mode=DR)
                # evictions for this chunk, split at the owned-rows
                # boundary so the accum covers exactly the owned segment
                for ti, (r0, g) in enumerate(grp):
                    n = g * Wp
                    ps3 = pss[(ti, oc)][:, :n].rearrange(
                        "p (r w) -> p r w", w=Wp)
                    segs = []
                    if r0 < 0:
                        segs.append((r0, min(r0 + g, 0), False))
                    o0, o1 = max(r0, 0), min(r0 + g, R)
                    if o0 < o1:
                        segs.append((o0, o1, True))
                    if r0 + g > R:
                        segs.append((max(r0, R), r0 + g, False))
                    for (s0, s1, own) in segs:
                        bsl = B[li][:, oc, s0 - lv.lo(0):s1 - lv.lo(0),
                                    1:1 + W]
                        psl = ps3[:, s0 - r0:s1 - r0, 1:1 + W]
                        if own:
                            t = tix[oc]
                            tix[oc] += 1
                            nc.vector.tensor_scalar(
                                out=bsl, in0=psl, scalar1=1.0, scalar2=0.0,
                                op0=ALU.mult, op1=ALU.add,
                                accum_out=pa[:, oc, t:t + 1])
                        else:
                            nc.vector.tensor_copy(out=bsl, in_=psl)
                emit_sq(oc, grp[-1][0] + grp[-1][1],
                        final=(gi + 3 >= len(tiles)))
        return pa, pb, nstat, max(nsq)

    def fold_apply(lv, j, pa, pb, nstat, nsq):
        """Fold stat partials -> per-group alpha/beta -> relu-apply B->A."""
        li = lv.idx
        W, R = lv.W, lv.R
        ninv = 1.0 / float(R * W)
        ninv2 = 1.0 / float(((R + 1) // 2) * W)
        cb2 = pct[:, 0, j, :]
        t12 = small.tile([128, 4], F32, name="t12", tag="t12")
        sa2 = small.tile([128, NCK], F32, name="sa2", tag="sa2")
        nc.vector.tensor_reduce(out=sa2, in_=pa[:, :, 0:nstat],
                                axis=mybir.AxisListType.X, op=ALU.add)
        sb2 = small.tile([128, NCK], F32, name="sb2", tag="sb2")
        nc.vector.tensor_reduce(out=sb2, in_=pb[:, :, 0:nsq],
                                axis=mybir.AxisListType.X, op=ALU.add)
        # t12[0:2] = E[z] per channel = sa/n + cb
        nc.vector.scalar_tensor_tensor(
            out=t12[:, 0:2], in0=sa2, scalar=ninv, in1=cb2,
            op0=ALU.mult, op1=ALU.add)
        # t12[2:4] = E[z^2] = sb/n2 + cb*(2*sa/n + cb)
        u = small.tile([128, NCK], F32, name="u", tag="u")
        nc.vector.scalar_tensor_tensor(
            out=u, in0=sa2, scalar=2.0 * ninv, in1=cb2,
            op0=ALU.mult, op1=ALU.add)
        w1 = small.tile([128, NCK], F32, name="w1", tag="w1")
        nc.vector.tensor_mul(out=w1, in0=u, in1=cb2)
        nc.vector.scalar_tensor_tensor(
            out=t12[:, 2:4], in0=sb2, scalar=ninv2, in1=w1,
            op0=ALU.mult, op1=ALU.add)
        # group sums via matmul with the 16-channel group mask
        gps = psS.tile([128, 4], F32, name="gps", tag="gps")
        nc.tensor.matmul(gps, gmt, t12, start=True, stop=True)
        me4 = small.tile([128, 4], F32, name="me4", tag="me4")
        nc.vector.tensor_scalar_mul(out=me4, in0=gps, scalar1=1.0 / 16.0)
        vr = small.tile([128, NCK], F32, name="vr", tag="vr")
        nc.vector.scalar_tensor_tensor(
            out=vr, in0=me4[:, 0:2], scalar=-1.0, in1=me4[:, 0:2],
            op0=ALU.mult, op1=ALU.mult)
        nc.vector.tensor_add(out=vr, in0=me4[:, 2:4], in1=vr)
        sd = small.tile([128, NCK], F32, name="sd", tag="sd")
        nc.scalar.activation(out=sd, in_=vr, func=AF.Sqrt, bias=epst,
                             scale=1.0)
        rstd = small.tile([128, NCK], F32, name="rstd", tag="rstd")
        nc.vector.reciprocal(out=rstd, in_=sd)
        al2 = small.tile([128, NCK], F32, name="al2", tag="al2")
        nc.vector.tensor_mul(out=al2, in0=pct[:, 1, j, :], in1=rstd)
        bt2 = small.tile([128, NCK], F32, name="bt2", tag="bt2")
        nc.vector.tensor_tensor(out=bt2, in0=cb2, in1=me4[:, 0:2],
                                op=ALU.subtract)
        be2 = small.tile([128, NCK], F32, name="be2", tag="be2")
        nc.vector.tensor_mul(out=be2, in0=bt2, in1=al2)
        nc.vector.tensor_add(out=be2, in0=be2, in1=pct[:, 2, j, :])
        # relu-apply; out rows [lo(j), hi(j)) of A. For split levels the
        # halo rows on the image-edge side must come out zero: use
        # mask-scaled coefficients (al*m, be*m with m in {0,1} per-core)
        # so the apply itself writes zeros there.
        lo, hi = lv.lo(j), lv.hi(j)

        def apply_rows(c0, c1, al, be, ck):
            if c1 <= c0:
                return
            nc.scalar.activation(
                out=A[li][:, ck, brow(lv, c0):brow(lv, c0) + (c1 - c0),
                          1:1 + W],
                in_=B[li][:, ck, c0 - lv.lo(0):c1 - lv.lo(0), 1:1 + W],
                func=AF.Relu, bias=be[:, ck:ck + 1],
                scale=al[:, ck:ck + 1])

        if lv.split:
            alt = small.tile([128, NCK], F32, name="alt", tag="alt")
            bet = small.tile([128, NCK], F32, name="bet", tag="bet")
            alb = small.tile([128, NCK], F32, name="alb", tag="alb")
            beb = small.tile([128, NCK], F32, name="beb", tag="beb")
            nc.vector.tensor_scalar_mul(out=alt, in0=al2,
                                        scalar1=mkt[:, li, 0:1])
            nc.vector.tensor_scalar_mul(out=bet, in0=be2,
                                        scalar1=mkt[:, li, 0:1])
            nc.vector.tensor_scalar_mul(out=alb, in0=al2,
                                        scalar1=mkt[:, li, 1:2])
            nc.vector.tensor_scalar_mul(out=beb, in0=be2,
                                        scalar1=mkt[:, li, 1:2])
        first = min(8, R)
        q = max(4, (R - first) // 3)
        for ck in range(NCK):
            if lv.split:
                apply_rows(lo, 0, alt, bet, ck)
            apply_rows(0, first, al2, be2, ck)
            for (c0, cn) in _row_tiles(first, R, q):
                apply_rows(c0, c0 + cn, al2, be2, ck)
            if lv.split:
                apply_rows(R, hi, alb, beb, ck)

    def head_level(lv):
        li = lv.idx
        Wp, W, R = lv.Wp, lv.W, lv.R
        Afl = A[li].rearrange("p c r w -> p c (r w)")
        hb = hpt[:, 0:1]
        mrelu = hpt[:, 1:2]
        inv_sh = hpt[:, 2:3]
        tiles = _row_tiles(0, R, lv.g_head)
        for gi in range(0, len(tiles), 3):
            grp = tiles[gi:gi + 3]
            pss = [psA.tile([HEAD_PAD, 512], F32, name="ps_head", tag="psa")
                   for _ in grp]
            for k in range(9):
                dy, dx = k // 3, k % 3
                sh = (dy - 1) * Wp + (dx - 1)
                lhsT = wht[:, k, :, 0:HEAD_PAD]
                for ti, (r0, g) in enumerate(grp):
                    n = g * Wp
                    base = brow(lv, r0) * Wp
                    rhs = Afl[:, :, base + sh: base + sh + n]
                    nc.tensor.matmul(pss[ti][:, :n], lhsT, rhs,
                                     start=(k == 0), stop=(k == 8),
                                     perf_mode=DR)
            for ti, (r0, g) in enumerate(grp):
                n = g * Wp
                hs = hstg.tile([HEAD_CH, lv.g_head * W], F32, name="hs",
                               tag="hs")
                ps3 = pss[ti][:HEAD_CH, :n].rearrange(
                    "p (r w) -> p r w", w=Wp)
                hs3 = hs[:, :g * W].rearrange("p (r w) -> p r w", w=W)
                # descale (1/s_head) + bias in one op
                nc.vector.tensor_scalar(
                    out=hs3, in0=ps3[:, :, 1:1 + W], scalar1=inv_sh,
                    scalar2=hb, op0=ALU.mult, op1=ALU.add)
                # selective relu: max(m*u, u); m=0 -> relu, m=1 -> identity
                nc.vector.scalar_tensor_tensor(
                    out=hs[:, :g * W], in0=hs[:, :g * W], scalar=mrelu,
                    in1=hs[:, :g * W], op0=ALU.mult, op1=ALU.max)
                px0 = OUT_BASE[li] + r0 * W
                nc.sync.dma_start(out=out[:, px0:px0 + g * W],
                                  in_=hs[:, :g * W])

    # ================= schedule =================
    # Interleave folds between convs so each level's alpha/beta (and the
    # first apply chunks) are ready before the tensor engine loops back
    # to that level at the next depth.
    for j in range(3):
        for lv in LEVELS:
            fold_apply(lv, j, *conv_level(lv, j))
    for lv in LEVELS:
        head_level(lv)

    ctx.close()


# ===================== host side =====================

_CACHE = {}
_last_results = None
FP8 = ml_dtypes.float8_e4m3


def _q8(x, scale=1.0):
    return (np.asarray(x, np.float32) * scale).astype(FP8)


def _wscale(w):
    m = float(np.abs(w).max())
    if m == 0:
        return 1.0
    return float(2.0 ** np.floor(np.log2(200.0 / m)))


def _pack_core(feats_q, tower_w, tower_b, gn_s, gn_b, sws,
               head_w, head_b, head_m, s_h, img, half):
    """Per-core input dict for one (img, branch, half)."""
    xin = np.zeros((128, XIN_COLS), FP8)
    for lv in LEVELS:
        f = feats_q[lv.idx][img]  # [256, H, W] fp8
        own0 = half * lv.R if lv.split else 0
        r_lo = own0 - lv.ht
        nin = lv.ht + lv.R + lv.ht
        for ck in range(NCK):
            blk = np.zeros((128, nin, lv.W), FP8)
            for r in range(nin):
                gr = r_lo + r
                if 0 <= gr < lv.H:
                    blk[:, r, :] = f[ck * 128:(ck + 1) * 128, gr, :]
            o = XIN_OFF[(lv.idx, ck)]
            xin[:, o:o + nin * lv.W] = blk.reshape(128, -1)

    # tower weights [3,256out,256in,3,3] -> [128ic, 3, 9, 2ick, 256oc] fp8
    wtp = np.zeros((128, 3, 9, NCK, CFPN), FP8)
    for j in range(3):
        w = tower_w[j] * sws[j]                      # [256o, 256i, 3, 3]
        w = np.transpose(w, (1, 2, 3, 0))            # [256i, 3, 3, 256o]
        w = w.reshape(NCK, 128, 9, CFPN)             # [ick, ic, 9, oc]
        wtp[:, j] = np.transpose(w, (1, 2, 0, 3)).astype(FP8)

    # head weights [85, 256, 3, 3] -> [128ic, 9, 2ick, 85] fp8
    hw = np.transpose(head_w * s_h, (1, 2, 3, 0)).reshape(
        NCK, 128, 9, HEAD_CH)
    whp = np.zeros((128, 9, NCK, CFPN), FP8)
    whp[:, :, :, :HEAD_CH] = np.transpose(hw, (1, 2, 0, 3)).astype(FP8)

    # per-channel params [128, {scaled bias, gn_s, gn_b}, 3 layer, 2 chunk]
    pcp = np.zeros((128, 3, 3, NCK), np.float32)
    for j in range(3):
        for ck in range(NCK):
            sl = slice(ck * 128, (ck + 1) * 128)
            pcp[:, 0, j, ck] = tower_b[j][sl] * sws[j]
            pcp[:, 1, j, ck] = gn_s[j][sl]
            pcp[:, 2, j, ck] = gn_b[j][sl]

    # edge-side halo masks: 0 on the image edge side, 1 on the halo side
    mkp = np.zeros((128, 5, 2), np.float32)
    for lv in LEVELS:
        if lv.split:
            mkp[:, lv.idx, 0] = 0.0 if half == 0 else 1.0
            mkp[:, lv.idx, 1] = 1.0 if half == 0 else 0.0

    hpp = np.zeros((HEAD_CH, 4), np.float32)
    hpp[:, 0] = head_b
    hpp[:, 1] = head_m
    hpp[:, 2] = 1.0 / s_h

    gmp = np.zeros((128, 128), np.float32)
    for i in range(128):
        gmp[i, (i // 16) * 16:(i // 16) * 16 + 16] = 1.0

    return {"xin": xin, "wt": wtp, "wh": whp, "pc": pcp, "mk": mkp,
            "hp": hpp, "gm": gmp}


def kernel(feat0, feat1, feat2, feat3, feat4,
           cls_conv_w, cls_conv_b, cls_gn_s, cls_gn_b, cls_out_w, cls_out_b,
           reg_conv_w, reg_conv_b, reg_gn_s, reg_gn_b,
           box_w, box_b, ctr_w, ctr_b):
    global _last_results
    feats = [np.asarray(f, np.float32) for f in
             (feat0, feat1, feat2, feat3, feat4)]
    feats_q = [f.astype(FP8) for f in feats]

    if "nc" not in _CACHE:
        _CACHE["nc"] = build_program()
        _CACHE["nc"].finalize()
    nc = _CACHE["nc"]

    # branch-specific head weights (padded to 85 out channels)
    allw = np.concatenate([np.asarray(cls_out_w, np.float32),
                           np.asarray(box_w, np.float32),
                           np.asarray(ctr_w, np.float32)], axis=0)
    allb = np.concatenate([np.asarray(cls_out_b, np.float32),
                           np.asarray(box_b, np.float32),
                           np.asarray(ctr_b, np.float32)])
    w_cls = allw.copy(); w_cls[80:] = 0.0
    w_reg = allw.copy(); w_reg[:80] = 0.0
    b_cls = allb.copy(); b_cls[80:] = 0.0
    b_reg = allb.copy(); b_reg[:80] = 0.0
    m_cls = np.ones(HEAD_CH, np.float32)
    m_reg = np.ones(HEAD_CH, np.float32)
    m_reg[80:84] = 0.0                       # relu on box channels

    branch_args = {}
    for br, (tw, tb, gs, gb, hw_, hb_, hm_) in {
        0: (cls_conv_w, cls_conv_b, cls_gn_s, cls_gn_b, w_cls, b_cls, m_cls),
        1: (reg_conv_w, reg_conv_b, reg_gn_s, reg_gn_b, w_reg, b_reg, m_reg),
    }.items():
        tw = np.asarray(tw, np.float32)
        sws = [_wscale(tw[j]) for j in range(3)]
        s_h = _wscale(hw_)
        branch_args[br] = (tw, np.asarray(tb, np.float32),
                           np.asarray(gs, np.float32),
                           np.asarray(gb, np.float32), sws,
                           hw_, hb_, hm_, s_h)

    in_maps = []
    for core in range(N_CORES):
        img = core // 4
        br = (core // 2) % 2
        half = core % 2
        tw, tb, gs, gb, sws, hw_, hb_, hm_, s_h = branch_args[br]
        in_maps.append(_pack_core(feats_q, tw, tb, gs, gb, sws,
                                  hw_, hb_, hm_, s_h, img, half))

    res = run_bass_kernel_spmd(nc, in_maps, core_ids=list(range(N_CORES)))
    _last_results = res

    fullout = np.zeros((N_BATCH, 20267, 85), np.float32)
    GBASE = {0: 0, 1: 15200, 2: 19000, 3: 19950, 4: 20197}
    for core in range(N_CORES):
        img = core // 4
        br = (core // 2) % 2
        half = core % 2
        ch = slice(0, 80) if br == 0 else slice(80, 85)
        o = np.asarray(res.results[core]["out"], np.float32)
        for lv in LEVELS:
            n = lv.R * lv.W
            src = o[ch, OUT_BASE[lv.idx]:OUT_BASE[lv.idx] + n].T
            if lv.split:
                d0 = GBASE[lv.idx] + half * n
                fullout[img, d0:d0 + n, ch] = src
            elif half == 0:
                fullout[img, GBASE[lv.idx]:GBASE[lv.idx] + n, ch] = src
    return fullout

